# revision 2
# baseline (speedup 1.0000x reference)
"""Distributed Trainium2 (Bass/Tile) kernel for an NT-Xent-style contrastive
loss over 8 NeuronCores.

Reference math:
    z       = row-normalized concat(emb_i, emb_j)          (2N, D), 2N=8192
    sim     = z @ z.T
    e       = exp(sim / T)
    denom_i = sum_{j != i} e_ij
    nom_i   = sum_{j != i, y_j == y_i} e_ij
    loss    = sum_i log(denom_i / nom_i) / (2 * 2N)

Strategy ("layered triangle", data-parallel over the sim matrix):

* Host prep (sharding): rows are sorted by label y (pure permutation - the
  loss is invariant), so every class owns exactly one contiguous 128-row
  block and the positive mask becomes block-diagonal.  Embeddings are
  L2-normalized, transposed to d-major (contraction dim on partitions) and
  cast to bf16.  Every core receives all 2N normalized embeddings
  (the "all-gather" of the sharding hint, realized as a replicated upload)
  plus a packed slab of its own 8 j-blocks.

* sim is symmetric, so each unordered block pair is computed ONCE:
  unit = (j-block jb: 128 rows on partitions) x (i-column IT: 1024 cols on
  the free dim), computed iff jb <= 8*IT+7.  Core c takes j-blocks
  {c, c+8, ..., c+8*IT} of every column IT - 36 units for every core, with
  identical compile-time structure (only the packed weight data differs,
  so one SPMD program serves all 8 cores).  Per unit:
    sim    = wblock.T @ z[:, IT-column]   (PE, 2x N=512 bf16 matmuls)
    E      = exp(sim / T)                 (ACT, one instr over both banks)
             + fused accum_out per-partition row sums = the free-direction
               S_all contribution of block jb (non-band units)
    colsum = ones32.T @ E                 (PE) accumulated in PSUM strips
             (tile_position col-groups pack 4 i-columns per PSUM tile; the
             band unit's colsum is kept separate - it is both the S_all
             partition-direction part and, restricted to the class block,
             the nominator S_pos).
  Strips are drained with one wide DVE copy per group and DMA'd out.

* Host gather: partial colsum/rowsum vectors from the 8 cores are summed
  into S_all/S_pos (each pair contributes exactly once by the band rule),
  the diagonal term exp(1/T) is dropped, and the final O(2N) log/mean gives
  the scalar loss.
"""

import numpy as np
import ml_dtypes

BS = 8192                   # 2N
D = 128                     # embedding dim == partition count
TEMPERATURE = 0.5
SCALE = 1.0 / TEMPERATURE
N_CORES = 8
ITW = 1024                  # i-column width
NIT = BS // ITW             # 8 i-columns
N_NONBAND = sum(range(NIT))  # 28 non-band units per core
SELF_E = float(np.exp(SCALE))

_NC_CACHE = {}


def _build_nc():
    if "nc" in _NC_CACHE:
        return _NC_CACHE["nc"]

    import concourse.tile as tile
    from concourse import bacc, mybir
    from concourse.bass import ts

    bf16 = mybir.dt.bfloat16
    f32 = mybir.dt.float32
    Exp = mybir.ActivationFunctionType.Exp

    nc = bacc.Bacc("TRN2", target_bir_lowering=False, debug=False,
                   num_devices=N_CORES)

    # all 2N normalized embeddings, d-major
    zT_d = nc.dram_tensor("zT", [D, BS], bf16, kind="ExternalInput")
    # this core's 8 j-blocks {c, c+8, ..., c+56}, packed
    w_d = nc.dram_tensor("w", [D, NIT * 128], bf16, kind="ExternalInput")

    # colsum rows; see drain DMAs for the row mapping
    out_cols_d = nc.dram_tensor("out_cols", [2 * NIT, ITW], f32,
                                kind="ExternalOutput")
    out_free_d = nc.dram_tensor("out_free", [128, N_NONBAND], f32,
                                kind="ExternalOutput")

    with tile.TileContext(nc) as tc:
        with (
            tc.tile_pool(name="w", bufs=1) as wp,
            tc.tile_pool(name="x", bufs=1) as xp,
            tc.tile_pool(name="ps", bufs=2, space="PSUM") as pp,
            tc.tile_pool(name="acc", bufs=1, space="PSUM") as accp,
            tc.tile_pool(name="e", bufs=4) as ep,
            tc.tile_pool(name="small", bufs=1) as smallp,
        ):
            ones_sb = smallp.tile([128, 32], bf16)
            nc.vector.memset(ones_sb[:], 1.0)

            w_sb = wp.tile([D, NIT * 128], bf16, tag="w")
            nc.sync.dma_start(w_sb[:], w_d[:])
            # whole zT in 4 chunk DMAs, last i-columns first (processing
            # order) - few dma_start calls = little first-byte latency.
            x_sb = xp.tile([D, BS], bf16, tag="x")
            for ch in range(3, -1, -1):
                nc.sync.dma_start(x_sb[:, ts(ch, 2048)], zT_d[:, ts(ch, 2048)])

            free_sb = smallp.tile([128, N_NONBAND], f32, tag="free")
            nfree = 0

            # Two halves of 4 i-columns each.  Within a half, column IT
            # accumulates its colsums into a 32-partition strip (strip g via
            # tile_position col-group g) of a shared PSUM tile, as 32
            # identical copies (M=32 all-ones weights) - one DVE copy drains
            # 4 columns at once; a strided DMA picks one row per strip.
            for half in range(2):
                its = range(7, 3, -1) if half == 0 else range(3, -1, -1)
                acc_t = accp.tile([128, ITW], f32, tag="acc_all")
                band_t = accp.tile([128, ITW], f32, tag="band_out")
                for IT in its:
                    # half 1 reverses strips so the never-written IT=0 acc
                    # strip sits at partitions 96.. (drain [0:96])
                    g = IT - 4 if half == 0 else 3 - IT
                    n_nb = IT  # non-band units in this column
                    for u in range(IT + 1):
                        band = u == IT
                        wk = w_sb[:, ts(u, 128)]
                        ps = pp.tile([128, ITW], f32, tag="ps")
                        for h in range(2):
                            nc.tensor.matmul(ps[:, ts(h, 512)], wk,
                                             x_sb[:, ts(2 * IT + h, 512)],
                                             start=True, stop=True)
                        e = ep.tile([128, ITW], bf16, tag="e")
                        if band:
                            nc.scalar.activation(e[:], ps[:], Exp, scale=SCALE)
                        else:
                            nc.scalar.activation(
                                e[:], ps[:], Exp, scale=SCALE,
                                accum_out=free_sb[:, nfree:nfree + 1])
                            nfree += 1
                        tgt = band_t if band else acc_t
                        for h in range(2):
                            nc.tensor.matmul(
                                tgt[32 * g:32 * g + 32, ts(h, 512)],
                                ones_sb[:], e[:, ts(h, 512)],
                                start=band or u == 0,
                                stop=band or u == n_nb - 1,
                                tile_position=(0, 32 * g))
                # drain the half: one wide DVE copy per accumulator tile
                asb = smallp.tile([128, ITW], f32, tag="acc_sb", bufs=2)
                bsb = smallp.tile([128, ITW], f32, tag="band_sb", bufs=2)
                if half == 0:
                    nc.vector.tensor_copy(asb[:], acc_t[:])
                    nc.sync.dma_start(out_cols_d[4:8, :], asb[0:128:32, :])
                    nc.vector.tensor_copy(bsb[:], band_t[:])
                    nc.sync.dma_start(out_cols_d[12:16, :], bsb[0:128:32, :])
                else:
                    # IT=0 has no non-band units: its (reversed) strip 3 of
                    # acc_t is never written; drain strips 0..2 only.
                    # Row mapping: acc row 4-IT (IT 3..1), band row 11-IT.
                    nc.vector.tensor_copy(asb[0:96, :], acc_t[0:96, :])
                    nc.sync.dma_start(out_cols_d[1:4, :], asb[0:96:32, :])
                    nc.vector.tensor_copy(bsb[:], band_t[:])
                    nc.sync.dma_start(out_cols_d[8:12, :], bsb[0:128:32, :])

            nc.sync.dma_start(out_free_d[:], free_sb[:])

    nc.compile()
    _NC_CACHE["nc"] = nc
    return nc


def _prep_in_maps(emb_i, emb_j, y):
    """Host-side sharding prep: normalize, label-sort, transpose, cast."""
    emb = np.concatenate([np.asarray(emb_i, np.float32),
                          np.asarray(emb_j, np.float32)], axis=0)
    z = emb / np.sqrt((emb * emb).sum(axis=1, keepdims=True))
    perm = np.argsort(np.asarray(y), kind="stable")  # class blocks of 128
    zs = z[perm]
    zT = np.ascontiguousarray(zs.T).astype(ml_dtypes.bfloat16)  # (D, 2N)
    in_maps = []
    for c in range(N_CORES):
        blocks = [zT[:, (c + 8 * b) * 128:(c + 8 * b + 1) * 128]
                  for b in range(NIT)]
        in_maps.append({
            "zT": zT,
            "w": np.ascontiguousarray(np.concatenate(blocks, axis=1)),
        })
    return in_maps


def _combine(results):
    """Gather/unshard: assemble S_all / S_pos from the per-core partials."""
    s_all = np.zeros(BS, np.float64)
    s_pos = np.zeros(BS, np.float64)
    acc_row = {IT: (IT if IT >= 4 else 4 - IT) for IT in range(1, NIT)}
    band_row = {IT: (8 + IT if IT >= 4 else 11 - IT) for IT in range(NIT)}
    for c, r in enumerate(results):
        cols = r["out_cols"].astype(np.float64)
        free = r["out_free"].astype(np.float64)
        nfree = 0
        for IT in range(NIT - 1, -1, -1):
            if IT > 0:
                s_all[IT * ITW:(IT + 1) * ITW] += cols[acc_row[IT]]
            s_all[IT * ITW:(IT + 1) * ITW] += cols[band_row[IT]]
            jb_band = c + 8 * IT
            s_pos[jb_band * 128:(jb_band + 1) * 128] = \
                cols[band_row[IT], c * 128:(c + 1) * 128]
            for u in range(IT):
                jb = c + 8 * u
                s_all[jb * 128:(jb + 1) * 128] += free[:, nfree]
                nfree += 1
    lp = np.log(s_all - SELF_E) - np.log(s_pos - SELF_E)
    return np.float32(lp.sum() / (2 * BS))


def kernel(emb_i, emb_j, y):
    from concourse.bass_utils import run_bass_kernel_spmd
    nc = _build_nc()
    in_maps = _prep_in_maps(emb_i, emb_j, y)
    res = run_bass_kernel_spmd(nc, in_maps, list(range(N_CORES)))
    return _combine(res.results)


def run_traced(emb_i, emb_j, y, **trace_kwargs):
    """kernel() with NTFF profiling where available; returns (loss, results)."""
    from concourse.bass_utils import run_bass_kernel_spmd
    nc = _build_nc()
    in_maps = _prep_in_maps(emb_i, emb_j, y)
    res = run_bass_kernel_spmd(nc, in_maps, list(range(N_CORES)), trace=True,
                               **trace_kwargs)
    return _combine(res.results), res


# revision 5
# speedup vs baseline: 1.0249x; 1.0249x over previous
"""Distributed Trainium2 (Bass/Tile) kernel for an NT-Xent-style contrastive
loss over 8 NeuronCores.

Reference math:
    z       = row-normalized concat(emb_i, emb_j)          (2N, D), 2N=8192
    sim     = z @ z.T
    e       = exp(sim / T)
    denom_i = sum_{j != i} e_ij
    nom_i   = sum_{j != i, y_j == y_i} e_ij
    loss    = sum_i log(denom_i / nom_i) / (2 * 2N)

Strategy ("layered triangle", data-parallel over the sim matrix):

* Host prep (sharding): rows are sorted by label y (pure permutation - the
  loss is invariant), so every class owns exactly one contiguous 128-row
  block and the positive mask becomes block-diagonal.  Embeddings are
  L2-normalized, transposed to d-major (contraction dim on partitions) and
  cast to bf16.  Every core receives all 2N normalized embeddings
  (the "all-gather" of the sharding hint, realized as a replicated upload)
  plus a packed slab of its own 8 j-blocks.

* sim is symmetric, so each unordered block pair is computed ONCE:
  unit = (j-block jb: 128 rows on partitions) x (i-column IT: 1024 cols on
  the free dim), computed iff jb <= 8*IT+7.  Core c takes j-blocks
  {c, c+8, ..., c+8*IT} of every column IT - 36 units for every core, with
  identical compile-time structure (only the packed weight data differs,
  so one SPMD program serves all 8 cores).  Per unit:
    sim    = wblock.T @ z[:, IT-column]   (PE, 2x N=512 bf16 matmuls)
    E      = exp(sim / T)                 (ACT, one instr over both banks)
             + fused accum_out per-partition row sums = the free-direction
               S_all contribution of block jb (non-band units)
    colsum = ones32.T @ E                 (PE) accumulated in PSUM strips
             (tile_position col-groups pack 4 i-columns per PSUM tile; the
             band unit's colsum is kept separate - it is both the S_all
             partition-direction part and, restricted to the class block,
             the nominator S_pos).
  Strips are drained with one wide DVE copy per group and DMA'd out.

* Host gather: partial colsum/rowsum vectors from the 8 cores are summed
  into S_all/S_pos (each pair contributes exactly once by the band rule),
  the diagonal term exp(1/T) is dropped, and the final O(2N) log/mean gives
  the scalar loss.
"""

import numpy as np
import ml_dtypes

BS = 8192                   # 2N
D = 128                     # embedding dim == partition count
TEMPERATURE = 0.5
SCALE = 1.0 / TEMPERATURE
N_CORES = 8
ITW = 1024                  # i-column width
NIT = BS // ITW             # 8 i-columns
N_NONBAND = sum(range(NIT))  # 28 non-band units per core
SELF_E = float(np.exp(SCALE))

_NC_CACHE = {}


def _build_nc():
    if "nc" in _NC_CACHE:
        return _NC_CACHE["nc"]

    import concourse.tile as tile
    from concourse import bacc, mybir
    from concourse.bass import ts

    bf16 = mybir.dt.bfloat16
    f32 = mybir.dt.float32
    Exp = mybir.ActivationFunctionType.Exp

    nc = bacc.Bacc("TRN2", target_bir_lowering=False, debug=False,
                   num_devices=N_CORES)

    # all 2N normalized embeddings, d-major
    zT_d = nc.dram_tensor("zT", [D, BS], bf16, kind="ExternalInput")
    # this core's 8 j-blocks {c, c+8, ..., c+56}, packed
    w_d = nc.dram_tensor("w", [D, NIT * 128], bf16, kind="ExternalInput")

    # colsum rows; see drain DMAs for the row mapping
    out_cols_d = nc.dram_tensor("out_cols", [2 * NIT, ITW], f32,
                                kind="ExternalOutput")
    out_free_d = nc.dram_tensor("out_free", [128, N_NONBAND], f32,
                                kind="ExternalOutput")

    with tile.TileContext(nc) as tc:
        with (
            tc.tile_pool(name="w", bufs=1) as wp,
            tc.tile_pool(name="x", bufs=1) as xp,
            tc.tile_pool(name="ps", bufs=2, space="PSUM") as pp,
            tc.tile_pool(name="acc", bufs=1, space="PSUM") as accp,
            tc.tile_pool(name="e", bufs=4) as ep,
            tc.tile_pool(name="small", bufs=1) as smallp,
        ):
            ones_sb = smallp.tile([128, 32], bf16)
            nc.vector.memset(ones_sb[:], 1.0)
            # warm the ACT exp spline table during the DMA lead-in so the
            # first real exp doesn't pay the ~2.7us ACT_TABLE_LOAD
            warm_sb = smallp.tile([128, 1], f32)
            nc.scalar.activation(warm_sb[:], ones_sb[:, 0:1], Exp)

            w_sb = wp.tile([D, NIT * 128], bf16, tag="w")
            nc.sync.dma_start(w_sb[:, 0:128], w_d[:, 0:128])
            # whole zT in chunk DMAs, last i-columns first (processing
            # order); the first unit's slab (IT=7) leads so PE starts as
            # early as possible.
            x_sb = xp.tile([D, BS], bf16, tag="x")
            nc.sync.dma_start(x_sb[:, 7168:7680], zT_d[:, 7168:7680])
            nc.sync.dma_start(x_sb[:, 7680:8192], zT_d[:, 7680:8192])
            nc.sync.dma_start(w_sb[:, 128:], w_d[:, 128:])
            nc.sync.dma_start(x_sb[:, 6144:7168], zT_d[:, 6144:7168])
            for ch in range(2, -1, -1):
                nc.sync.dma_start(x_sb[:, ts(ch, 2048)], zT_d[:, ts(ch, 2048)])

            free_sb = smallp.tile([128, N_NONBAND], f32, tag="free")
            nfree = 0

            # Two halves of 4 i-columns each.  Within a half, column IT
            # accumulates its colsums into a 32-partition strip (strip g via
            # tile_position col-group g) of a shared PSUM tile, as 32
            # identical copies (M=32 all-ones weights) - one DVE copy drains
            # 4 columns at once; a strided DMA picks one row per strip.
            for half in range(2):
                its = range(7, 3, -1) if half == 0 else range(3, -1, -1)
                acc_t = accp.tile([128, ITW], f32, tag="acc_all")
                band_t = accp.tile([128, ITW], f32, tag="band_out")
                for IT in its:
                    # half 1 reverses strips so the never-written IT=0 acc
                    # strip sits at partitions 96.. (drain [0:96])
                    g = IT - 4 if half == 0 else 3 - IT
                    n_nb = IT  # non-band units in this column
                    for u in range(IT + 1):
                        band = u == IT
                        wk = w_sb[:, ts(u, 128)]
                        ps = pp.tile([128, ITW], f32, tag="ps")
                        for h in range(2):
                            nc.tensor.matmul(ps[:, ts(h, 512)], wk,
                                             x_sb[:, ts(2 * IT + h, 512)],
                                             start=True, stop=True)
                        e = ep.tile([128, ITW], bf16, tag="e")
                        if band:
                            nc.scalar.activation(e[:], ps[:], Exp, scale=SCALE)
                        else:
                            nc.scalar.activation(
                                e[:], ps[:], Exp, scale=SCALE,
                                accum_out=free_sb[:, nfree:nfree + 1])
                            nfree += 1
                        tgt = band_t if band else acc_t
                        for h in range(2):
                            nc.tensor.matmul(
                                tgt[32 * g:32 * g + 32, ts(h, 512)],
                                ones_sb[:], e[:, ts(h, 512)],
                                start=band or u == 0,
                                stop=band or u == n_nb - 1,
                                tile_position=(0, 32 * g))
                # drain the half: one wide DVE copy per accumulator tile
                asb = smallp.tile([128, ITW], f32, tag="acc_sb", bufs=2)
                bsb = smallp.tile([128, ITW], f32, tag="band_sb", bufs=2)
                if half == 0:
                    nc.vector.tensor_copy(asb[:], acc_t[:])
                    nc.sync.dma_start(out_cols_d[4:8, :], asb[0:128:32, :])
                    nc.vector.tensor_copy(bsb[:], band_t[:])
                    nc.sync.dma_start(out_cols_d[12:16, :], bsb[0:128:32, :])
                else:
                    # IT=0 has no non-band units: its (reversed) strip 3 of
                    # acc_t is never written; drain strips 0..2 only.
                    # Row mapping: acc row 4-IT (IT 3..1), band row 11-IT.
                    nc.vector.tensor_copy(asb[0:96, :], acc_t[0:96, :])
                    nc.sync.dma_start(out_cols_d[1:4, :], asb[0:96:32, :])
                    nc.vector.tensor_copy(bsb[:], band_t[:])
                    nc.sync.dma_start(out_cols_d[8:12, :], bsb[0:128:32, :])

            nc.sync.dma_start(out_free_d[:], free_sb[:])

    nc.compile()
    _NC_CACHE["nc"] = nc
    return nc


def _prep_in_maps(emb_i, emb_j, y):
    """Host-side sharding prep: normalize, label-sort, transpose, cast."""
    emb = np.concatenate([np.asarray(emb_i, np.float32),
                          np.asarray(emb_j, np.float32)], axis=0)
    z = emb / np.sqrt((emb * emb).sum(axis=1, keepdims=True))
    perm = np.argsort(np.asarray(y), kind="stable")  # class blocks of 128
    zs = z[perm]
    zT = np.ascontiguousarray(zs.T).astype(ml_dtypes.bfloat16)  # (D, 2N)
    in_maps = []
    for c in range(N_CORES):
        blocks = [zT[:, (c + 8 * b) * 128:(c + 8 * b + 1) * 128]
                  for b in range(NIT)]
        in_maps.append({
            "zT": zT,
            "w": np.ascontiguousarray(np.concatenate(blocks, axis=1)),
        })
    return in_maps


def _combine(results):
    """Gather/unshard: assemble S_all / S_pos from the per-core partials."""
    s_all = np.zeros(BS, np.float64)
    s_pos = np.zeros(BS, np.float64)
    acc_row = {IT: (IT if IT >= 4 else 4 - IT) for IT in range(1, NIT)}
    band_row = {IT: (8 + IT if IT >= 4 else 11 - IT) for IT in range(NIT)}
    for c, r in enumerate(results):
        cols = r["out_cols"].astype(np.float64)
        free = r["out_free"].astype(np.float64)
        nfree = 0
        for IT in range(NIT - 1, -1, -1):
            if IT > 0:
                s_all[IT * ITW:(IT + 1) * ITW] += cols[acc_row[IT]]
            s_all[IT * ITW:(IT + 1) * ITW] += cols[band_row[IT]]
            jb_band = c + 8 * IT
            s_pos[jb_band * 128:(jb_band + 1) * 128] = \
                cols[band_row[IT], c * 128:(c + 1) * 128]
            for u in range(IT):
                jb = c + 8 * u
                s_all[jb * 128:(jb + 1) * 128] += free[:, nfree]
                nfree += 1
    lp = np.log(s_all - SELF_E) - np.log(s_pos - SELF_E)
    return np.float32(lp.sum() / (2 * BS))


def kernel(emb_i, emb_j, y):
    from concourse.bass_utils import run_bass_kernel_spmd
    nc = _build_nc()
    in_maps = _prep_in_maps(emb_i, emb_j, y)
    res = run_bass_kernel_spmd(nc, in_maps, list(range(N_CORES)))
    return _combine(res.results)


def run_traced(emb_i, emb_j, y, **trace_kwargs):
    """kernel() with NTFF profiling where available; returns (loss, results)."""
    from concourse.bass_utils import run_bass_kernel_spmd
    nc = _build_nc()
    in_maps = _prep_in_maps(emb_i, emb_j, y)
    res = run_bass_kernel_spmd(nc, in_maps, list(range(N_CORES)), trace=True,
                               **trace_kwargs)
    return _combine(res.results), res
